# revision 64
# baseline (speedup 1.0000x reference)
"""Multi-head attention + layernorm Bass kernel for Trainium2, 8 cores.

Problem: B=8, S=1024, D=768, H=12 heads x DH=64, key-padding mask, softmax,
output projection, layernorm.  Sharding: pure data parallelism - one batch
element per NeuronCore, no collectives.

Design (bf16 matmuls, fp32 psum; ACT-exp ~100us and PE ~183us busy are the
engine budgets; measured ~224-227us, was 247us):
  - bf16 (not fp16) matmul operands: same PE rate, but lower multiplier
    power -> the power throttle relaxes (50%-util-limit windows 29.5%->
    24.3% of runtime, avg util limit 78%->83%), worth ~4us end-to-end.
    Accuracy cost ~2^-8: rel err 1.3e-3 -> 3.7e-3, still 5x under the
    2e-2 gate.
  - braided emission: the in-order PE stream interleaves dependency-free
    work units into the exp-paced scores loop; three queues drained in
    priority order -- next pair's projections FIRST (so qt/kt are ready
    before the phase boundary; flush_p() before each scores phase is the
    deadlock guard), then V units (deps always ready), then ctx/norm/out.
    fill(1) and fill(3) both REGRESS (backlog shifts the tail / slots
    underfill).  out(3) queues into scores(5,1)'s exp-paced stalls.
  - ~130 warmup matmuls cover the 6.6us NEFF prologue + input-DMA window
    (DMA-bound to ~15us; do NOT delay wk0 -- kt(0) gates the first
    scores matmul, reordering it after xt cost ~8us).
  - braid norm chains run at hp650 so they outrank LN stats (hp600) in
    the DVE stream: ctx(p,i) blocks on the 1-deep pc ring until
    norm(p-1,i) drains, which had PE stalling ~2us at the braid end.
  - weights prescaled x64 on host; the scale cancels through the softmax
    normalize and folds into the exp scale 2^-15 (q also carries 1/sqrt(dh)).
  - iblk-outer attention; out-proj blocks s0..3 ride inside the second-iblk
    braid.  Tail (all after the last exp, PE-saturated): out(3) on the pa
    ring (pb would chain it to the last exp via pst slot reuse), ctx(4,1)
    on pc + ctx(5,1) on a dead pb pst slot run back-to-back, then both
    normalizes, out4/out5 accumulate pairs 0..4 while they drain, single
    [128,768] psum tiles for out5/6/7 on early-freeing pb/pc/pb slots
    (depth-4 effective; z_sb bufs=4 so z6/z7 don't wait on out-DMAs).
  - softmax denominators: ones-column in V -> psum row 64 (psum reads must
    start quadrant-aligned, so ones stays LAST); copy row (ACT identity in
    the tail, DVE in the braid), reciprocal_approx_fast + cast in 512-col
    halves (single-partition ops are ~1.3ns/elem serial), K=1 bf16 matmul
    broadcast, fused psum->ct normalize multiplies.
  - LN: bn_stats in one 512- + one 256-wide window (BN_STATS_FMAX=512;
    aggr merges unequal windows via group counts); rstd via quake seed + 2 Newton steps on DVE in the
    braid; ACT Sqrt + DVE recip in the tail (ACT Rsqrt/Reciprocal are
    BLOCKED by bass for accuracy).  z-scale on ACT for s>=3.
  - gamma==1/beta==0 (true for this problem instance) selects a program
    variant that skips the gamma/beta ops entirely; the general variant
    keeps them (gpsimd braid / DVE tail).
  - all input DMAs on the sync queue ordered by first use.

Residuals at ~225us (roadmap):
  - 7.5us fixed NEFF/runtime prologue before the first instruction.
  - ~6 x 1us ctx-vs-exp waits mid-braid: the exp stream (ends ~189us)
    is paced by fill(2) slot spacing; a true exp-lead braid remains the
    big structural win (~10-15us) but naive fill changes regress --
    would need PE-time-budgeted filling per slot.
  - e_p bufs=28 overflows SBUF (z_p needs 45.75KB); 24 is the max.
  - a DVE TT with two PSUM operands fails BIR verification (the pb16
    cast stays).  out7 as two psum tiles (for earlier LN stats) REGRESSES
    ~9us: the pb-ring resequencing chains out7b to out5's late z-read.
  - tail after last mm ~10us: LN6/LN7 chains + last DMA + ~3us drains.
  - moving q/k bias to ACT identity REGRESSES (delays next exps); gpsimd
    cannot read psum (V-bias must stay on DVE).
  - fp8 fails the 2e-2 gate everywhere in this net: every path is a
    random-sign average, so fp8's ~2-3% relative noise survives at full
    strength (measured 4-7% for attention-path fp8).  bf16 z/output
    DMA also REGRESSES ~8us (and costs +1e-3 err).
"""

import numpy as np

B, S, D, H, DH = 8, 1024, 768, 12, 64
NPAIR, NQUAD = H // 2, H // 4
SBLK = S // 128      # 8 key/row chunks
DCH = D // 128       # 6 contraction chunks
LN_EPS = 1e-5
NEG_MASK = -30.0
W64 = 64.0           # host weight prescale
EXP_SCALE = 1.0 / (64.0 * 64.0 * 8.0)   # qt64*kt64 -> scores/8
VW = 65              # per-head stride in V layout: [v64, ones]
VQW = 4 * VW         # 260, per-quad width
FP8 = False
N_WARM = 130

_PROGRAMS = {}


def _build_program(apply_gb=True):
    import concourse.bass as bass
    from concourse import bacc
    import concourse.tile as tile
    import concourse.mybir as mybir
    from contextlib import ExitStack

    F32 = mybir.dt.float32
    F16 = mybir.dt.bfloat16
    F8 = mybir.dt.float8e4
    FA = F8 if FP8 else F16
    DR = mybir.MatmulPerfMode.DoubleRow if FP8 else None
    AF = mybir.ActivationFunctionType
    CP = 2 if FP8 else 1          # contraction chunks consumed per matmul

    nc = bacc.Bacc("TRN2", target_bir_lowering=False)

    xt_d = nc.dram_tensor("xt", [128, DCH * S], FA, kind="ExternalInput")
    wq_d = nc.dram_tensor("wq", [NPAIR, 128, DCH * 128], FA, kind="ExternalInput")
    wk_d = nc.dram_tensor("wk", [NPAIR, 128, DCH * 128], FA, kind="ExternalInput")
    wv_d = nc.dram_tensor("wv", [NQUAD, 128, DCH * VQW], FA, kind="ExternalInput")
    wo_d = nc.dram_tensor("wo", [128, DCH * D], F16, kind="ExternalInput")
    bqk_d = nc.dram_tensor("bqk", [128, 2 * NPAIR], F32, kind="ExternalInput")
    bv_d = nc.dram_tensor("bv", [1, NQUAD * VQW], F32, kind="ExternalInput")
    maskb_d = nc.dram_tensor("maskb", [128, SBLK], F32, kind="ExternalInput")
    gamma_d = nc.dram_tensor("gamma", [1, D], F32, kind="ExternalInput")
    beta_d = nc.dram_tensor("beta", [1, D], F32, kind="ExternalInput")
    ones_d = nc.dram_tensor("ones16", [1, 128], F16, kind="ExternalInput")
    onesr_d = nc.dram_tensor("onesr", [1, 128], mybir.dt.float32r,
                             kind="ExternalInput")
    bo_d = nc.dram_tensor("bo16", [1, D], F16, kind="ExternalInput")
    out_d = nc.dram_tensor("out", [S, D], F32, kind="ExternalOutput")

    # j -> (et group, slot in group); groups pair key-chunks for DoubleRow
    ET_SLOT = [(0, 0), (0, 1), (3, 0), (1, 0), (1, 1), (3, 1), (2, 0), (2, 1)]
    # group -> (v dim1 slice start, stop, step)
    GRP_V = {0: (0, 2, 1), 1: (3, 5, 1), 2: (6, 8, 1), 3: (2, 6, 3)}

    with tile.TileContext(nc) as tc, ExitStack() as ctx:
        const = ctx.enter_context(tc.tile_pool(name="const", bufs=1))
        xt_p = ctx.enter_context(tc.tile_pool(name="xt_p", bufs=1))
        w_p = ctx.enter_context(tc.tile_pool(name="w_p", bufs=1))
        qk_p = ctx.enter_context(tc.tile_pool(name="qk_p", bufs=1))
        v_p = ctx.enter_context(tc.tile_pool(name="v_p", bufs=1))
        e_p = ctx.enter_context(tc.tile_pool(name="e_p", bufs=1))
        cx_p = ctx.enter_context(tc.tile_pool(name="cx_p", bufs=1))
        z_p = ctx.enter_context(tc.tile_pool(name="z_p", bufs=1))
        ps = ctx.enter_context(tc.tile_pool(name="ps", bufs=1, space="PSUM"))

        # ---- warmup stationary (DVE memset, no DMA dependency) ----
        warm16 = const.tile([128, 64], F16)
        nc.vector.memset(warm16, 0.25)

        # ---- input DMAs, all on the sync queue: ordered so the first
        # projection (wq0/wk0 + xt) and first exp (bqk, maskb) unblock ASAP
        # three xt tiles so readers of early chunks don't wait on the
        # whole 1.6MB transfer (one tile = one dependency unit in Tile)
        xts = [xt_p.tile([128, 2, S], FA, name="xt%d" % i) for i in range(3)]
        wq_ts = [w_p.tile([128, DCH, 128], FA, name="wqp", bufs=NPAIR)
                 for _ in range(NPAIR)]
        wk_ts = [w_p.tile([128, DCH, 128], FA, name="wkp", bufs=NPAIR)
                 for _ in range(NPAIR)]
        wv_ts = [w_p.tile([128, DCH, VQW], FA, name="wvq", bufs=NQUAD)
                 for _ in range(NQUAD)]
        bqk_t = const.tile([128, 2 * NPAIR], F32)
        mask_t = const.tile([128, SBLK], F32)
        bv_t = const.tile([128, NQUAD * VQW], F32)
        ones_t = const.tile([1, 128], F16)
        onesr_t = const.tile([1, 128], mybir.dt.float32r)
        bo_t = const.tile([1, D], F16)
        gamma_t = const.tile([128, D], F32)
        beta_t = const.tile([128, D], F32)
        woa = w_p.tile([128, DCH, D], F16, name="woa", bufs=1)

        nc.sync.dma_start(out=wq_ts[0], in_=wq_d[0])
        nc.sync.dma_start(out=wk_ts[0], in_=wk_d[0])
        nc.sync.dma_start(out=xts[0], in_=xt_d[:, 0:2 * S])
        nc.sync.dma_start(out=xts[1], in_=xt_d[:, 2 * S:4 * S])
        nc.sync.dma_start(out=xts[2], in_=xt_d[:, 4 * S:6 * S])
        nc.sync.dma_start(out=bqk_t, in_=bqk_d[:, :])
        nc.sync.dma_start(out=mask_t, in_=maskb_d[:, :])
        nc.sync.dma_start(out=wq_ts[1], in_=wq_d[1])
        nc.sync.dma_start(out=wk_ts[1], in_=wk_d[1])
        nc.sync.dma_start(out=ones_t, in_=ones_d[:, :])
        nc.sync.dma_start(out=onesr_t, in_=onesr_d[:, :])
        nc.sync.dma_start(out=bo_t, in_=bo_d[:, :])
        nc.sync.dma_start(out=wq_ts[2], in_=wq_d[2])
        nc.sync.dma_start(out=wk_ts[2], in_=wk_d[2])
        nc.sync.dma_start(out=wv_ts[0], in_=wv_d[0])
        nc.sync.dma_start(out=bv_t, in_=bv_d[0:1, :].to_broadcast([128, NQUAD * VQW]))
        for p in range(3, NPAIR):
            nc.sync.dma_start(out=wq_ts[p], in_=wq_d[p])
            nc.sync.dma_start(out=wk_ts[p], in_=wk_d[p])
        for q in range(1, NQUAD):
            nc.sync.dma_start(out=wv_ts[q], in_=wv_d[q])
        nc.sync.dma_start(out=woa, in_=wo_d[:, :])
        if apply_gb:
            nc.sync.dma_start(out=gamma_t,
                              in_=gamma_d[0:1, :].to_broadcast([128, D]))
            nc.sync.dma_start(out=beta_t,
                              in_=beta_d[0:1, :].to_broadcast([128, D]))
        eps_t = const.tile([128, 1], F32)
        nc.vector.memset(eps_t, LN_EPS)
        magic_t = const.tile([128, 1], mybir.dt.int32)
        nc.vector.memset(magic_t, 0x5F3759DF)

        # ---- PE warmup: keep HAM busy during input DMA ----
        pw = ps.tile([64, 64], F32, name="pw", tag="pa", bufs=2,
                     padded_shape=[128, 512])
        for _ in range(N_WARM):
            nc.tensor.matmul(pw, warm16, warm16, start=True, stop=True)

        # ---- emit helpers ----
        v8 = [v_p.tile([128, SBLK, VQW], FA, name="v8", bufs=NQUAD)
              for _ in range(NQUAD)]
        qt = [qk_p.tile([128, S], F16, name="qt", bufs=NPAIR) for _ in range(NPAIR)]
        kt = [qk_p.tile([128, S], F16, name="kt", bufs=NPAIR) for _ in range(NPAIR)]
        ct = [cx_p.tile([128, S], F16, name="ct", bufs=NPAIR) for _ in range(NPAIR)]

        def emit_vquad(q):
            wv_t = wv_ts[q]

            def mk_v(s):
                def f():
                    psv = ps.tile([128, 512], F32, name="psv", tag="pa",
                                  bufs=2)
                    for ci in range(DCH // CP):
                        nc.tensor.matmul(
                            psv[:, 0:VQW],
                            xts[ci // 2][:, ci % 2,
                                         s * 128:(s + 1) * 128],
                            wv_t[:, ci, :],
                            start=(ci == 0), stop=(ci == DCH - 1))
                    with tc.high_priority(offset=350):
                        nc.vector.tensor_add(
                            out=v8[q][:, s, :], in0=psv[:, 0:VQW],
                            in1=bv_t[:, q * VQW:(q + 1) * VQW])
                return f

            for s in range(SBLK):
                pending_v.append((1000, mk_v(s)))

        def emit_proj(p, queued=False):
            def mk_half(dst, w_t, bcol, half):
                def f():
                    psq = ps.tile([128, 512], F32, name="psq", tag="pa",
                                  bufs=2)
                    for ci in range(DCH):
                        nc.tensor.matmul(
                            psq,
                            w_t[:, ci, :],
                            xts[ci // 2][:, ci % 2,
                                         half * 512:(half + 1) * 512],
                            start=(ci == 0), stop=(ci == DCH - 1))
                    with tc.high_priority(offset=400):
                        nc.vector.tensor_scalar_add(
                            out=dst[:, half * 512:(half + 1) * 512], in0=psq,
                            scalar1=bqk_t[:, bcol:bcol + 1])
                return f

            for dst, w_t, bcol in ((qt[p], wq_ts[p], p),
                                   (kt[p], wk_ts[p], NPAIR + p)):
                for half in range(2):
                    u = mk_half(dst, w_t, bcol, half)
                    if queued:
                        pending_p.append((1550, u))
                    else:
                        u()

        ET = {}
        from collections import deque
        pending = deque()
        pending_v = deque()   # V blocks: deps always ready, drain first
        pending_p = deque()   # next pair's projections: drain FIRST so
                              # qt/kt are ready before the phase boundary

        # queue entries are (measured_pe_ns, fn): drain by real PE
        # time (calibrated from traces, incl. issue overheads) at the
        # conservation rate ~1.65us/slot, smoothing both dry-spell PE
        # stalls and burst overdrains that stretch the exp stream
        debt = [0]

        def fill(n, budget=1650):
            debt[0] = min(debt[0] + budget, 3300)
            while debt[0] > 0 and (pending_p or pending_v or pending):
                if pending_p:
                    c, f = pending_p.popleft()
                elif pending_v:
                    c, f = pending_v.popleft()
                else:
                    c, f = pending.popleft()
                f()
                debt[0] -= c

        def flush_p():
            # safety: projections MUST be emitted before the scores phase
            # that reads them (in-order PE would deadlock otherwise)
            while pending_p:
                pending_p.popleft()[1]()

        def flush():
            flush_p()
            while pending_v:
                pending_v.popleft()[1]()
            while pending:
                pending.popleft()[1]()

        def emit_scores(p, iblk):
            # scores + exp feed ACT; after each slot, drain two queued
            # dependency-free PE work units so the in-order PE never idles
            ets = []
            for j in range(SBLK):
                pst = ps.tile([128, 1024], F32, name="pst", tag="pb", bufs=2)
                nc.tensor.matmul(
                    pst[:, 0:512], kt[p][0:64, j * 128:(j + 1) * 128],
                    qt[p][0:64, iblk * 512:(iblk + 1) * 512],
                    start=True, stop=True, tile_position=(0, 0))
                nc.tensor.matmul(
                    pst[:, 512:1024], kt[p][64:128, j * 128:(j + 1) * 128],
                    qt[p][64:128, iblk * 512:(iblk + 1) * 512],
                    start=True, stop=True, tile_position=(64, 0))
                et = e_p.tile([128, 1024], FA, name="et", bufs=24)
                nc.scalar.activation(et, pst, AF.Exp,
                                     bias=mask_t[:, j:j + 1],
                                     scale=EXP_SCALE)
                ets.append(et)
                fill(2)
            ET[(p, iblk)] = ets

        def emit_ctx(p, iblk, direct=False, psum_tag="pc"):
            qx = 2 * p // 4
            l0 = (2 * p) % 4
            ets = ET.pop((p, iblk))
            box = []

            def mk_ctx(j):
                def f():
                    if j == 0:
                        box.append(ps.tile([65, 1024], F32, name="pcx",
                                           tag=psum_tag,
                                           bufs=1 if psum_tag == "pc" else 2))
                    pcx = box[0]
                    for idx in range(2):
                        nc.tensor.matmul(
                            pcx[0:65, idx * 512:(idx + 1) * 512],
                            v8[qx][:, j,
                                   (l0 + idx) * VW:(l0 + idx + 1) * VW],
                            ets[j][:, idx * 512:(idx + 1) * 512],
                            start=(j == 0), stop=(j == SBLK - 1))
                return f

            def norm(act_copy=False):
                # denominator row (psum partition 64) -> reciprocal ->
                # fp16 -> K=1 broadcast matmul; runs in 512-col halves to
                # cut the serial single-partition latency.  Tail norms put
                # the psum-row copy on ACT (idle once the exps are done).
                # Braid norms gate ctx via the 1-deep pc ring, so their
                # DVE ops outrank LN stats (hp600) in the DVE stream.
                hp = 300 if act_copy else 650
                pcx = box[0]
                rxs = z_p.tile([1, 1024], F32, name="rxs", bufs=3)
                rx = z_p.tile([1, 1024], F32, name="rx", bufs=3)
                rx16 = z_p.tile([1, 1024], F16, name="rx16", bufs=3)
                pbc = ps.tile([128, 512], F32, name="pbc", tag="pa", bufs=2)
                for h in range(2):
                    sl = slice(h * 512, (h + 1) * 512)
                    if act_copy:
                        # tail: psum-row copy on ACT (idle post-exp)
                        nc.scalar.activation(rxs[:, sl], pcx[64:65, sl],
                                             AF.Identity)
                        nc.vector.reciprocal_approx_fast(out=rx[:, sl],
                                                         in_=rxs[:, sl])
                        nc.vector.tensor_copy(out=rx16[:, sl],
                                              in_=rx[:, sl])
                    else:
                        with tc.high_priority(offset=hp):
                            nc.vector.tensor_copy(out=rxs[:, sl],
                                                  in_=pcx[64:65, sl])
                            nc.vector.reciprocal_approx_fast(
                                out=rx[:, sl], in_=rxs[:, sl])
                            nc.vector.tensor_copy(out=rx16[:, sl],
                                                  in_=rx[:, sl])
                    if h == 0:
                        nc.tensor.matmul(pbc[0:64, :], ones_t[0:1, 0:64],
                                         rx16[0:1, sl],
                                         start=True, stop=True)
                    else:
                        nc.tensor.matmul(pbc[64:128, :], ones_t[0:1, 0:64],
                                         rx16[0:1, sl],
                                         start=True, stop=True,
                                         tile_position=(0, 64))
                # (a TT with two PSUM operands fails BIR verification, so
                # the broadcast goes through one sbuf fp16 cast)
                pb16 = z_p.tile([128, 512], F16, name="pb16", bufs=3)
                with tc.high_priority(offset=0 if act_copy else hp):
                    nc.vector.tensor_copy(out=pb16, in_=pbc)
                    nc.vector.tensor_mul(
                        out=ct[p][0:64, iblk * 512:(iblk + 1) * 512],
                        in0=pcx[0:64, 0:512], in1=pb16[0:64, :])
                    nc.vector.tensor_mul(
                        out=ct[p][64:128, iblk * 512:(iblk + 1) * 512],
                        in0=pcx[0:64, 512:1024], in1=pb16[64:128, :])

            if direct:
                for j in range(SBLK):
                    mk_ctx(j)()
                return norm
            for j in range(SBLK):
                pending.append((650, mk_ctx(j)))
            pending.append((200, norm))

        def emit_out(s, direct=True, ring=None, split=False, single=None,
                     fine=False):
            # alternate psum rings so consecutive out-blocks never wait on
            # each other's LN drain (depth-2 pipeline in the tail); tail
            # blocks 5/6/7 instead take a SINGLE [128,768] tile on a ring
            # slot that frees early (pb: last exp / norm drains, pc:
            # norm(4,1) drain) so their matmuls never wait on LN z-reads
            ring_ = ring or ("pa" if s % 2 == 0 else "pb")
            box = {}

            def acc(key, d0, dn, c0, c1, bias):
                if single is not None:
                    if "s" not in box:
                        box["s"] = ps.tile(
                            [128, 768], F32, name="pso_s", tag=single,
                            bufs=1 if single == "pc" else 2)
                    pt = box["s"][:, d0:d0 + dn]
                else:
                    if key not in box:
                        box[key] = ps.tile([128, 512], F32,
                                           name="pso_" + key,
                                           tag=ring_, bufs=2)
                    pt = box[key][:, 0:dn]
                for c in range(c0, c1):
                    nc.tensor.matmul(
                        pt,
                        ct[c][:, s * 128:(s + 1) * 128],
                        woa[:, c, d0:d0 + dn],
                        start=(c == 0), stop=False)
                if bias:
                    nc.tensor.matmul(pt, ones_t,
                                     bo_t[0:1, d0:d0 + dn],
                                     start=False, stop=True)

            def ln():
                if single is not None:
                    emit_ln(s, box["s"][:, 0:512], box["s"][:, 512:768],
                            whole=box["s"])
                else:
                    emit_ln(s, box["a"], box["b"])

            if split:
                # accumulate pairs 0..4 now (ct[5] not ready yet); the
                # finisher adds pair 5 + bias and runs LN
                acc("a", 0, 512, 0, NPAIR - 1, False)
                acc("b", 512, 256, 0, NPAIR - 1, False)

                def fin():
                    acc("a", 0, 512, NPAIR - 1, NPAIR, True)
                    acc("b", 512, 256, NPAIR - 1, NPAIR, True)
                    ln()
                return fin
            if fine and not direct:
                # braid blocks: ~1.3us sub-units so a drained unit never
                # delays the next pst matmul (exp pacing) by more than
                # half an out-projection
                pending.extend([
                    (1800, lambda: acc("a", 0, 512, 0, 3, False)),
                    (1800, lambda: acc("a", 0, 512, 3, NPAIR, True)),
                    (1400, lambda: acc("b", 512, 256, 0, 3, False)),
                    (1400, lambda: acc("b", 512, 256, 3, NPAIR, True)),
                    (50, ln)])
                return
            units = [(2900, lambda: acc("a", 0, 512, 0, NPAIR, True)),
                     (2100, lambda: acc("b", 512, 256, 0, NPAIR, True)),
                     (50, ln)]
            if direct:
                for _, u in units:
                    u()
            else:
                pending.extend(units)

        def emit_ln(s, pso_a, pso_b, whole=None):
            # BN_STATS_FMAX=512: one 512-wide + one 256-wide window; the
            # aggregate merges unequal windows via the per-group counts
            stats = z_p.tile([128, 2, 6], F32, name="stats", bufs=2)
            with tc.high_priority(offset=600):
                nc.vector.bn_stats(out=stats[:, 0, :], in_=pso_a[:, 0:512])
                nc.vector.bn_stats(out=stats[:, 1, :], in_=pso_b[:, 0:256])
                mv = z_p.tile([128, 2], F32, name="mv", bufs=2)
                nc.vector.bn_aggr(out=mv, in_=stats)
            I32 = mybir.dt.int32
            with tc.high_priority(offset=600):
                veps = z_p.tile([128, 1], F32, name="veps", bufs=2)
                nc.vector.tensor_scalar_add(out=veps, in0=mv[:, 1:2],
                                            scalar1=LN_EPS)
                if s >= 4:
                    # tail: sqrt on the (idle) ACT table engine + one DVE
                    # reciprocal replaces the 7-op Newton chain
                    sq = z_p.tile([128, 1], F32, name="sq", bufs=2)
                    nc.scalar.activation(sq, veps, AF.Sqrt)
                    rstd = z_p.tile([128, 1], F32, name="rstd", bufs=2)
                    nc.vector.reciprocal_approx_fast(out=rstd, in_=sq)
                else:
                    # braid: quake seed + 2 Newton steps, all on DVE --
                    # keeps the ACT engine exp-only (no table thrash)
                    hb = z_p.tile([128, 1], I32, name="hb", bufs=2)
                    nc.vector.tensor_scalar(
                        out=hb, in0=veps.bitcast(I32), scalar1=1,
                        scalar2=None,
                        op0=mybir.AluOpType.arith_shift_right)
                    y0 = z_p.tile([128, 1], I32, name="y0", bufs=2)
                    nc.vector.tensor_tensor(out=y0, in0=magic_t, in1=hb,
                                            op=mybir.AluOpType.subtract)
                    rstd = y0.bitcast(F32)
                    vm = z_p.tile([128, 1], F32, name="vm", bufs=2)
                    nc.vector.tensor_scalar_mul(out=vm, in0=veps,
                                                scalar1=-0.5)
                    tq = z_p.tile([128, 1], F32, name="tq", bufs=2)
                    for _ in range(2):
                        # 2 Newton steps: rstd rel err ~5e-6
                        nc.vector.tensor_mul(out=tq, in0=rstd, in1=rstd)
                        nc.vector.tensor_scalar(out=tq, in0=tq, scalar1=vm,
                                                scalar2=1.5,
                                                op0=mybir.AluOpType.mult,
                                                op1=mybir.AluOpType.add)
                        nc.vector.tensor_mul(out=y0.bitcast(F32), in0=rstd,
                                             in1=tq)
                nmr = z_p.tile([128, 1], F32, name="nmr", bufs=2)
                nc.vector.tensor_scalar(out=nmr, in0=mv[:, 0:1], scalar1=rstd,
                                        scalar2=-1.0, op0=mybir.AluOpType.mult,
                                        op1=mybir.AluOpType.mult)
                z = z_p.tile([128, D], F32, name="z_sb", bufs=4)
                if s >= 3:
                    # tail: ACT is idle once the exps are done
                    if whole is not None and s == SBLK - 1 \
                            and not apply_gb:
                        # last block: halves, so the final DMA issues
                        # while the second identity still runs
                        srcs = [(whole[:, 0:512], 0, 512),
                                (whole[:, 512:768], 512, 256)]
                        for src, d0, dn in srcs:
                            nc.scalar.activation(z[:, d0:d0 + dn], src,
                                                 AF.Identity, bias=nmr,
                                                 scale=rstd)
                            nc.sync.dma_start(
                                out=out_d[s * 128:(s + 1) * 128,
                                          d0:d0 + dn],
                                in_=z[:, d0:d0 + dn])
                        return
                    if whole is not None:
                        nc.scalar.activation(z, whole, AF.Identity,
                                             bias=nmr, scale=rstd)
                    else:
                        nc.scalar.activation(z[:, 0:512], pso_a, AF.Identity,
                                             bias=nmr, scale=rstd)
                        nc.scalar.activation(z[:, 512:768], pso_b[:, 0:256],
                                             AF.Identity, bias=nmr,
                                             scale=rstd)
                else:
                    nc.vector.tensor_scalar(out=z[:, 0:512], in0=pso_a,
                                            scalar1=rstd, scalar2=nmr,
                                            op0=mybir.AluOpType.mult,
                                            op1=mybir.AluOpType.add)
                    nc.vector.tensor_scalar(out=z[:, 512:768],
                                            in0=pso_b[:, 0:256],
                                            scalar1=rstd, scalar2=nmr,
                                            op0=mybir.AluOpType.mult,
                                            op1=mybir.AluOpType.add)
            if not apply_gb:
                # gamma==1, beta==0 for this problem instance: z IS the
                # final output
                nc.sync.dma_start(out=out_d[s * 128:(s + 1) * 128, :], in_=z)
                return
            zf = z_p.tile([128, D], F32, name="zf", bufs=2)
            if s >= 6:
                # last blocks: keep the chain on DVE (gpsimd TT is 1.8us/op
                # and fully exposed at the end)
                nc.vector.tensor_mul(out=z, in0=z, in1=gamma_t)
                nc.vector.tensor_add(out=zf, in0=z, in1=beta_t)
            else:
                nc.gpsimd.tensor_mul(out=z, in0=z, in1=gamma_t)
                nc.gpsimd.tensor_add(out=zf, in0=z, in1=beta_t)
            nc.sync.dma_start(out=out_d[s * 128:(s + 1) * 128, :], in_=zf)

        # ---- braided emission: scores slots drain queued dep-free PE
        # units (ctx of earlier pairs, V blocks, out blocks) so the
        # in-order PE stream never idles on exp-paced dependencies ----
        emit_proj(0)
        emit_vquad(0)
        emit_proj(1, queued=True)
        emit_scores(0, 0)
        flush_p()
        emit_proj(2, queued=True)
        emit_ctx(0, 0)
        emit_scores(1, 0)
        flush_p()
        emit_proj(3, queued=True)
        emit_vquad(1)
        emit_scores(2, 0)
        flush_p()
        emit_proj(4, queued=True)
        emit_ctx(1, 0)
        emit_scores(3, 0)
        flush_p()
        emit_proj(5, queued=True)
        emit_vquad(2)
        emit_ctx(2, 0)
        emit_scores(4, 0)
        flush_p()
        emit_ctx(3, 0)
        emit_scores(5, 0)
        emit_ctx(4, 0)
        emit_scores(0, 1)
        emit_ctx(5, 0)
        # out(0..2) need only iblk0 cts (ready after norm(5,0), which is
        # FIFO-ahead of them); queuing them a phase earlier spreads their
        # drains into (1,1)..(3,1) and relieves the (4,1)/(5,1) crunch
        emit_out(0, direct=False, fine=True)
        emit_scores(1, 1)
        emit_ctx(0, 1)
        emit_out(1, direct=False, fine=True)
        emit_scores(2, 1)
        emit_ctx(1, 1)
        emit_out(2, direct=False, fine=True)
        emit_scores(3, 1)
        emit_ctx(2, 1)
        emit_scores(4, 1)
        emit_ctx(3, 1)
        emit_out(3, direct=False, ring="pa")
        emit_scores(5, 1)
        flush()
        # ---- tail: out(3) queued into scores(5,1)'s exp-paced stall
        # windows, on the pa ring (pb would chain it to the last exp via
        # pst slot reuse); ctx(4,1)+ctx(5,1) matmuls back-to-back
        # (ctx(5,1) borrows a pst slot from the now-dead pb ring instead
        # of waiting for norm(4,1) to drain the 1-deep pc ring); out 4/5
        # accumulate pairs 0..4 while the last normalizes run ----
        n41 = emit_ctx(4, 1, direct=True)
        n51 = emit_ctx(5, 1, direct=True, psum_tag="pb")
        n41(act_copy=True)
        n51(act_copy=True)
        f4 = emit_out(4, split=True)
        f5 = emit_out(5, split=True, single="pb")
        f4()
        f5()
        emit_out(6, single="pc")
        emit_out(7, single="pb")

    nc.compile()
    return nc


def _np_f8():
    import ml_dtypes
    return ml_dtypes.float8_e4m3fn


def _np_bf16():
    import ml_dtypes
    return ml_dtypes.bfloat16


def _host_inputs(inputs):
    x = np.asarray(inputs["input_tensor"], np.float32)
    mask = np.asarray(inputs["attention_mask"])
    Wq = np.asarray(inputs["Wq"], np.float32)
    bq = np.asarray(inputs["bq"], np.float32)
    Wk = np.asarray(inputs["Wk"], np.float32)
    bk = np.asarray(inputs["bk"], np.float32)
    Wv = np.asarray(inputs["Wv"], np.float32)
    bv = np.asarray(inputs["bv"], np.float32)
    Wo = np.asarray(inputs["Wo"], np.float32)
    bo = np.asarray(inputs["bo"], np.float32)
    gamma = np.asarray(inputs["gamma"], np.float32)
    beta = np.asarray(inputs["beta"], np.float32)

    fa = _np_f8() if FP8 else _np_bf16()

    wq_flat = np.ascontiguousarray(Wq.transpose(1, 0, 2).reshape(D, D)) * W64
    wk_flat = np.ascontiguousarray(Wk.transpose(1, 0, 2).reshape(D, D)) * W64
    bq_s = bq.reshape(D) * W64
    bk_s = bk.reshape(D) * W64

    # ones column LAST per head: denominator lands at psum partition 64
    # (psum reads must start at a quadrant-aligned partition, so v-dims
    # stay at partitions 0..63)
    wv_aug = np.zeros((D, NQUAD * VQW), np.float32)
    bv_aug = np.zeros((1, NQUAD * VQW), np.float32)
    for h in range(H):
        q, l = divmod(h, 4)
        base = q * VQW + l * VW
        wv_aug[:, base:base + 64] = Wv[h] * W64
        bv_aug[0, base:base + 64] = bv[h] * W64
        bv_aug[0, base + 64] = W64

    bqk = np.zeros((128, 2 * NPAIR), np.float32)
    for p in range(NPAIR):
        bqk[:, p] = bq_s[p * 128:(p + 1) * 128]
        bqk[:, NPAIR + p] = bk_s[p * 128:(p + 1) * 128]

    def sbuf_layout(w, width, dt):
        # [D, n*width] -> [n, 128, DCH*width]: partition-major per tile
        n = w.shape[1] // width
        return np.ascontiguousarray(
            w.reshape(DCH, 128, n, width).transpose(2, 1, 0, 3).reshape(
                n, 128, DCH * width).astype(dt))

    shared = {
        "wq": sbuf_layout(wq_flat, 128, fa),
        "wk": sbuf_layout(wk_flat, 128, fa),
        "wv": sbuf_layout(wv_aug, VQW, fa),
        "wo": sbuf_layout(np.ascontiguousarray(Wo), D, _np_bf16())[0],
        "bqk": bqk, "bv": bv_aug,
        "gamma": gamma.reshape(1, D).copy(),
        "beta": beta.reshape(1, D).copy(),
        "ones16": np.ones((1, 128), _np_bf16()),
        "onesr": np.ones((1, 128), np.float32),
        "bo16": bo.reshape(1, D).astype(_np_bf16()),
    }
    in_maps = []
    for b in range(B):
        mb = np.where(mask[b], 0.0, NEG_MASK).astype(np.float32)
        in_maps.append({
            **shared,
            "xt": np.ascontiguousarray(
                x[b].T.reshape(DCH, 128, S).transpose(1, 0, 2).reshape(
                    128, DCH * S).astype(fa)),
            "maskb": np.ascontiguousarray(mb.reshape(SBLK, 128).T),
        })
    return in_maps


def _get_program(apply_gb=True):
    if apply_gb not in _PROGRAMS:
        _PROGRAMS[apply_gb] = _build_program(apply_gb)
    return _PROGRAMS[apply_gb]


def kernel(**inputs):
    from concourse.bass_utils import run_bass_kernel_spmd

    apply_gb = not (
        np.all(np.asarray(inputs["gamma"], np.float32) == 1.0)
        and np.all(np.asarray(inputs["beta"], np.float32) == 0.0))
    nc = _get_program(apply_gb)
    in_maps = _host_inputs(inputs)
    res = run_bass_kernel_spmd(nc, in_maps, list(range(B)))
    return np.stack([res.results[b]["out"] for b in range(B)],
                    axis=0).astype(np.float32)


if __name__ == "__main__":
    rng = np.random.default_rng(0)
    demo = {
        "input_tensor": rng.standard_normal((B, S, D)).astype(np.float32),
        "attention_mask": np.ones((B, S), bool),
        "Wq": (rng.standard_normal((H, D, DH)) * 0.03).astype(np.float32),
        "bq": (rng.standard_normal((H, DH)) * 0.03).astype(np.float32),
        "Wk": (rng.standard_normal((H, D, DH)) * 0.03).astype(np.float32),
        "bk": (rng.standard_normal((H, DH)) * 0.03).astype(np.float32),
        "Wv": (rng.standard_normal((H, D, DH)) * 0.03).astype(np.float32),
        "bv": (rng.standard_normal((H, DH)) * 0.03).astype(np.float32),
        "Wo": (rng.standard_normal((D, D)) * 0.03).astype(np.float32),
        "bo": (rng.standard_normal((D,)) * 0.03).astype(np.float32),
        "gamma": np.ones((D,), np.float32),
        "beta": np.zeros((D,), np.float32),
    }
    out = kernel(**demo)
    print("kernel ran, out shape", out.shape, "finite:", np.isfinite(out).all())

